# revision 1
# baseline (speedup 1.0000x reference)
"""GCNCheb Trainium2 kernel: out[b,n,fo] = sum_k T_k[b,n,:] @ W[k] + bias.

T_k recurrence (matrix powers P_j = L^j x with T0=P0, T1=P1, Tk=2*P_k - T_{k-2})
is linear, so the K/F_in contraction is re-expressed over pure powers with
host-precombined weights V_j:
    out = P0 (W0-W2) + P1 (W1-W3) + P2 (2 W2) + P3 (2 W3) + bias

Distribution over 8 NeuronCores: 1D row-shard of L. Core r holds the column
slice Lc_r = L[:, r*1024:(r+1)*1024] (== L[rows_r,:].T since L is symmetric),
which is exactly the lhsT operand the PE wants. X is [N, B*F_in] = [8192, 128]
(batch folded into columns). Each step every core computes its 1024 rows of
X_next = L @ X via 64 k-tile accumulations, then an AllGather rebuilds the full
X for the next step. Only 2 in-kernel AllGathers (4 MB each); the final output
is gathered host-side from the per-core results.

The projection runs on-device on the core's row shard: P_j^T tiles (built with
PE transposes) are contracted with block-diagonal weights packing all 4 batches,
producing out^T per core; the host untangles layout and adds bias.
"""

import os
import sys

sys.path.insert(0, "/opt/trn_rl_repo")

import numpy as np

import concourse.bass as bass
import concourse.mybir as mybir
import concourse.tile as tile
from concourse import bacc, bass_utils
from concourse.masks import make_identity

B, N, F_IN, F_OUT, K = 4, 8192, 32, 64, 4
NCORES = 8
P = 128
SH = N // NCORES          # rows per core (1024)
BF = B * F_IN             # folded X columns (128)
KT = N // P               # contraction tiles (64)
MT = SH // P              # output row tiles per core (8)
QH = 2                    # output halves: (b in {2h, 2h+1}) x F_OUT = 128 partitions

# "bf16": L/X matmuls in bf16 (L fully SBUF-resident, single DMA pass).
# "fp32": everything fp32 (L streamed from HBM each of the 3 steps).
VARIANT = os.environ.get("GCN_VARIANT", "bf16")

_DT = {"bf16": mybir.dt.bfloat16, "fp32": mybir.dt.float32}
_NPDT = {"bf16": "bfloat16", "fp32": "float32"}


def _np_dt(variant):
    if variant == "bf16":
        import ml_dtypes

        return np.dtype(ml_dtypes.bfloat16)
    return np.dtype(np.float32)


def build_nc(variant=VARIANT):
    dt = _DT[variant]
    f32 = mybir.dt.float32
    resident = variant == "bf16"  # 16 MB bf16 L fits in SBUF

    nc = bacc.Bacc()
    Lc = nc.dram_tensor("Lc", [N, SH], dt, kind="ExternalInput")
    X0 = nc.dram_tensor("X0", [N, BF], dt, kind="ExternalInput")
    X0T = nc.dram_tensor("X0T", [BF, SH], f32, kind="ExternalInput")
    WH = nc.dram_tensor("WH", [K, QH, BF, P], f32, kind="ExternalInput")
    OUT = nc.dram_tensor("OUT", [QH, P, SH], f32, kind="ExternalOutput")

    Lc3 = Lc.rearrange("(kt p) m -> p kt m", p=P)      # [128, KT, SH]
    X03 = X0.rearrange("(kt p) f -> p kt f", p=P)      # [128, KT, BF]

    with tile.TileContext(nc) as tc:
        with (
            tc.tile_pool(name="lres", bufs=1) as lres_pool,
            tc.tile_pool(name="lstream", bufs=4) as lstream_pool,
            tc.tile_pool(name="xbuf", bufs=1) as x_pool,
            tc.tile_pool(name="ybuf", bufs=2) as y_pool,
            tc.tile_pool(name="proj", bufs=1) as proj_pool,
            tc.tile_pool(name="psum", bufs=1, space="PSUM") as psum_pool,
            tc.tile_pool(name="dram", bufs=1, space="DRAM") as dram_pool,
        ):
            # --- constant loads ---
            whs = proj_pool.tile([P, K, QH, P], f32, tag="whs")
            nc.sync.dma_start(whs[:], WH.rearrange("k h p m -> p k h m"))
            pt0 = proj_pool.tile([P, SH], f32, tag="pt0")
            nc.sync.dma_start(pt0[:], X0T[:, :])
            ident = proj_pool.tile([P, P], f32, tag="ident")
            make_identity(nc, ident[:])

            if resident:
                lc_res = lres_pool.tile([P, KT, SH], dt, tag="lc_res")
                # chunked loads so DMA overlaps the early matmuls
                for ko in range(0, KT, 8):
                    nc.sync.dma_start(
                        lc_res[:, ko : ko + 8, :], Lc3[:, ko : ko + 8, :]
                    )

            x_cur = x_pool.tile([P, KT, BF], dt, tag="x")
            nc.sync.dma_start(x_cur[:], X03[:, :, :])

            pt = [pt0, None, None, None]

            for step in (1, 2, 3):
                # --- Y = (L @ X)[rows_r] : 8 concurrent psum accumulations ---
                ypsum = [
                    psum_pool.tile([P, BF], f32, tag=f"ps{mt}", name=f"y{step}_{mt}")
                    for mt in range(MT)
                ]
                if resident:
                    for kt in range(KT):
                        for mt in range(MT):
                            nc.tensor.matmul(
                                ypsum[mt][:],
                                lhsT=lc_res[:, kt, mt * P : (mt + 1) * P],
                                rhs=x_cur[:, kt, :],
                                start=(kt == 0),
                                stop=(kt == KT - 1),
                            )
                else:
                    KB = 2
                    for ko in range(0, KT, KB):
                        lc_t = lstream_pool.tile(
                            [P, KB, SH], dt, tag="lc_s", name=f"lc{step}_{ko}"
                        )
                        nc.sync.dma_start(lc_t[:], Lc3[:, ko : ko + KB, :])
                        for kb in range(KB):
                            kt = ko + kb
                            for mt in range(MT):
                                nc.tensor.matmul(
                                    ypsum[mt][:],
                                    lhsT=lc_t[:, kb, mt * P : (mt + 1) * P],
                                    rhs=x_cur[:, kt, :],
                                    start=(kt == 0),
                                    stop=(kt == KT - 1),
                                )

                # evacuate PSUM: fp32 copy for the projection path (+ dt copy
                # for the gather/matmul path when dt != fp32)
                ysh32 = y_pool.tile([P, MT, BF], f32, tag="ysh32", name=f"ysh32_{step}")
                for mt in range(MT):
                    nc.vector.tensor_copy(ysh32[:, mt, :], ypsum[mt][:])

                if step < 3:
                    if dt == f32:
                        yshd = ysh32
                    else:
                        yshd = y_pool.tile(
                            [P, MT, BF], dt, tag="yshd", name=f"yshd_{step}"
                        )
                        for mt in range(MT):
                            nc.vector.tensor_copy(yshd[:, mt, :], ypsum[mt][:])

                    shard = dram_pool.tile([SH, BF], dt, name=f"shard{step}")
                    full = dram_pool.tile(
                        [N, BF], dt, addr_space="Shared", name=f"full{step}"
                    )
                    nc.sync.dma_start(
                        shard.opt().rearrange("(mt p) f -> p mt f", p=P), yshd[:]
                    )
                    nc.gpsimd.collective_compute(
                        "AllGather",
                        mybir.AluOpType.bypass,
                        replica_groups=[list(range(NCORES))],
                        ins=[shard.opt()],
                        outs=[full.opt()],
                    )
                    x_cur = x_pool.tile([P, KT, BF], dt, tag="x", name=f"x{step}")
                    nc.sync.dma_start(
                        x_cur[:], full.opt().rearrange("(kt p) f -> p kt f", p=P)
                    )

                # --- transpose row shard for the projection: P_j^T [BF, SH] ---
                ptj = proj_pool.tile([P, SH], f32, tag=f"pt{step}", name=f"pt{step}")
                for mt in range(MT):
                    tp = psum_pool.tile(
                        [P, P], f32, tag=f"ps{mt}", name=f"tp{step}_{mt}"
                    )
                    nc.tensor.transpose(tp[:], ysh32[:, mt, :], ident[:])
                    nc.vector.tensor_copy(ptj[:, mt * P : (mt + 1) * P], tp[:])
                pt[step] = ptj

            # --- projection: outT[h] = sum_j WH[j,h].T @ P_j^T  (+ host bias) ---
            out_sb = proj_pool.tile([P, QH, 2, 512], f32, tag="out_sb")
            for h in range(QH):
                for ns in range(2):
                    pp = psum_pool.tile(
                        [P, 512], f32, tag=f"ps{h * 2 + ns}", name=f"pp{h}_{ns}"
                    )
                    for j in range(K):
                        nc.tensor.matmul(
                            pp[:],
                            lhsT=whs[:, j, h, :],
                            rhs=pt[j][:, ns * 512 : (ns + 1) * 512],
                            start=(j == 0),
                            stop=(j == K - 1),
                        )
                    nc.vector.tensor_copy(out_sb[:, h, ns, :], pp[:])
            nc.sync.dma_start(
                OUT.rearrange("h q (s n) -> q h s n", s=2), out_sb[:]
            )

    nc.compile()
    return nc


_CACHED = {}


def _get_nc(variant):
    if variant not in _CACHED:
        _CACHED[variant] = build_nc(variant)
    return _CACHED[variant]


def _prep_inputs(x, L, weight, variant):
    np_dt = _np_dt(variant)
    f32 = np.float32

    X0 = np.ascontiguousarray(
        x.astype(f32).transpose(1, 0, 2).reshape(N, BF)
    )  # [N, (b,fi)]
    W = weight.astype(f32)
    V = np.stack(
        [W[0] - W[2], W[1] - W[3], 2.0 * W[2], 2.0 * W[3]]
    )  # [4, F_IN, F_OUT]
    # block-diagonal packing: WH[j, h, b*F_IN+fi, bl*F_OUT+fo] = V[j,fi,fo]
    # for b == 2h + bl
    WH = np.zeros((K, QH, BF, P), dtype=f32)
    for j in range(K):
        for b in range(B):
            h, bl = divmod(b, 2)
            WH[j, h, b * F_IN : (b + 1) * F_IN, bl * F_OUT : (bl + 1) * F_OUT] = V[j]

    in_maps = []
    for r in range(NCORES):
        rows = slice(r * SH, (r + 1) * SH)
        Lc_r = np.ascontiguousarray(L[:, rows]).astype(np_dt)
        X0T_r = np.ascontiguousarray(X0[rows, :].T)
        in_maps.append(
            {
                "Lc": Lc_r,
                "X0": X0.astype(np_dt),
                "X0T": X0T_r,
                "WH": WH,
            }
        )
    return in_maps


def _assemble(results, bias):
    out = np.empty((B, N, F_OUT), dtype=np.float32)
    for r in range(NCORES):
        outT = results[r]["OUT"]  # [QH, 128, SH]
        for b in range(B):
            h, bl = divmod(b, 2)
            out[b, r * SH : (r + 1) * SH, :] = outT[
                h, bl * F_OUT : (bl + 1) * F_OUT, :
            ].T
    out += bias.astype(np.float32)
    return out


def run(x, L, weight, bias, variant=VARIANT, trace=False):
    nc = _get_nc(variant)
    in_maps = _prep_inputs(x, L, weight, variant)
    res = bass_utils.run_bass_kernel_spmd(
        nc,
        in_maps,
        core_ids=list(range(NCORES)),
        trace=trace,
        trace_cores=list(range(NCORES)) if trace else None,
    )
    out = _assemble(res.results, bias)
    return out, res


def kernel(x, L, weight, bias):
    out, _ = run(
        np.asarray(x), np.asarray(L), np.asarray(weight), np.asarray(bias)
    )
    return out


# revision 14
# speedup vs baseline: 2.7382x; 2.7382x over previous
"""GCNCheb Trainium2 kernel: out[b,n,fo] = sum_k T_k[b,n,:] @ W[k] + bias.

T_k recurrence (matrix powers P_j = L^j x with T0=P0, T1=P1, Tk=2*P_k - T_{k-2})
is linear, so the K/F_in contraction is re-expressed over pure powers with
host-precombined weights V_j:
    out = P0 (W0-W2) + P1 (W1-W3) + P2 (2 W2) + P3 (2 W3) + bias

Distribution over 8 NeuronCores: 1D row-shard of L. Core r holds the column
slice Lc_r = L[:, r*1024:(r+1)*1024] (== L[rows_r,:].T since L is symmetric),
pre-tiled on host to [128, 64, 1024] so every DMA is contiguous per partition.
X is [N, B*F_in] = [8192, 128] (batch folded into columns), pre-tiled to
[128, 64, 128]. Each step every core computes its 1024 rows of X_next = L @ X
via 64 k-tile PSUM accumulations, then an AllGather rebuilds the full X.

Latency structure: steps 1 and 2 are split into two m-half phases; each
half-shard is gathered as soon as its phase completes, so the collective's
firmware latency and inter-core skew hide behind the other half's matmuls.
Consuming steps issue k-tiles of gather-half A before half B. The final output
is gathered host-side from per-core results.

The projection runs on-device on the core's row shard: P_j^T tiles (built with
PE transposes) are contracted with block-diagonal weights packing all 4 batches,
producing out^T per core; the host untangles layout and adds bias.

bf16 variant (default): L is bf16 and fully SBUF-resident (16 MB), single DMA
pass split across both HWDGE queues (sync + scalar); projection path in bf16.
"""

import os
import sys

sys.path.insert(0, "/opt/trn_rl_repo")

import numpy as np

import concourse.bass as bass
import concourse.mybir as mybir
import concourse.tile as tile
from concourse import bacc, bass_utils
from concourse.masks import make_identity

B, N, F_IN, F_OUT, K = 4, 8192, 32, 64, 4
NCORES = 8
P = 128
SH = N // NCORES          # rows per core (1024)
BF = B * F_IN             # folded X columns (128)
KT = N // P               # contraction tiles (64)
MT = SH // P              # output row tiles per core (8)
MH = MT // 2              # half-shard m-tiles (4)
QH = 2                    # output halves: (b in {2h, 2h+1}) x F_OUT = 128 partitions

VARIANT = os.environ.get("GCN_VARIANT", "bf16")

_DT = {"bf16": mybir.dt.bfloat16, "fp32": mybir.dt.float32}


def _np_dt(variant):
    if variant == "bf16":
        import ml_dtypes

        return np.dtype(ml_dtypes.bfloat16)
    return np.dtype(np.float32)


def build_nc(variant=VARIANT):
    dt = _DT[variant]
    f32 = mybir.dt.float32
    resident = variant == "bf16"  # 16 MB bf16 L fits in SBUF

    nc = bacc.Bacc()
    # all pre-tiled on host: partition-major, fully contiguous per partition
    Lc = nc.dram_tensor("Lc", [4, P, KT, SH // 4], dt, kind="ExternalInput")
    X0 = nc.dram_tensor("X0", [P, KT, BF], dt, kind="ExternalInput")
    X0T = nc.dram_tensor("X0T", [BF, SH], dt, kind="ExternalInput")
    WH = nc.dram_tensor("WH", [K, QH, BF, P], dt, kind="ExternalInput")
    OUT = nc.dram_tensor("OUT", [QH, P, SH], f32, kind="ExternalOutput")

    # k-tile consumption orders matching how the previous step's gathers land
    def kts_of(mt0, nmt):
        return [r * MT + mt0 + m for r in range(NCORES) for m in range(nmt)]

    kt_quarters = sum([kts_of(2 * q, 2) for q in range(4)], [])
    kt_halves = kts_of(0, MH) + kts_of(MH, MH)

    with tile.TileContext(nc) as tc:
        with (
            tc.tile_pool(name="lres", bufs=1) as lres_pool,
            tc.tile_pool(name="lstream", bufs=6) as lstream_pool,
            tc.tile_pool(name="xbuf", bufs=1) as x_pool,
            tc.tile_pool(name="ybuf", bufs=2) as y_pool,
            tc.tile_pool(name="proj", bufs=1) as proj_pool,
            tc.tile_pool(name="psum", bufs=1, space="PSUM") as psum_pool,
            tc.tile_pool(name="dram", bufs=1, space="DRAM") as dram_pool,
        ):
            # --- initial loads: X first (everything waits on it), L chunks
            # alternating across the two HWDGE queues (sync + scalar) ---
            x_cur = x_pool.tile([P, KT, BF], dt, tag="x", name="x0")
            half = KT // 2
            nc.sync.dma_start(x_cur[:, :half, :], X0[:, :half, :])
            nc.sync.dma_start(x_cur[:, half:, :], X0[:, half:, :])

            LCH = 4  # k-tiles per L DMA chunk (fp32 streaming path)
            SH4 = SH // 4
            if resident:
                lc_res = lres_pool.tile([P, 4, KT, SH4], dt, tag="lc_res")
                for q in range(4):
                    for ko in range(0, KT, 8):
                        nc.scalar.dma_start(
                            lc_res[:, q, ko : ko + 8, :],
                            Lc[q, :, ko : ko + 8, :],
                        )

            whs = proj_pool.tile([P, K, QH, P], dt, tag="whs")
            nc.sync.dma_start(whs[:], WH.rearrange("k h p m -> p k h m"))
            pt0 = proj_pool.tile([P, SH], dt, tag="pt0")
            nc.sync.dma_start(pt0[:], X0T[:, :])
            ident = proj_pool.tile([P, P], dt, tag="ident")
            make_identity(nc, ident[:])

            pt = [pt0, None, None, None]

            def gather_slice(step, mt0, nmt, yshd, x_nxt):
                """DMA mt-slice of the shard out, AllGather it, DMA back."""
                shard = dram_pool.tile([P, nmt, BF], dt, name=f"shard{step}_{mt0}")
                full = dram_pool.tile(
                    [NCORES * P, nmt, BF],
                    dt,
                    addr_space="Shared",
                    name=f"full{step}_{mt0}",
                )
                nc.sync.dma_start(shard.opt(), yshd[:, mt0 : mt0 + nmt, :])
                nc.gpsimd.collective_compute(
                    "AllGather",
                    mybir.AluOpType.bypass,
                    replica_groups=[list(range(NCORES))],
                    ins=[shard.opt()],
                    outs=[full.opt()],
                )
                xv = x_nxt[:].rearrange("p (r mt) f -> p r mt f", r=NCORES)
                nc.scalar.dma_start(
                    xv[:, :, mt0 : mt0 + nmt, :],
                    full[:].rearrange("(r p) mt f -> p r mt f", p=P),
                )

            def transposes(step, yshd, n_mt=MT):
                """PE-transpose the row shard into P_j^T [BF, SH] for projection."""
                if pt[step] is None:
                    pt[step] = proj_pool.tile(
                        [P, SH], dt, tag=f"pt{step}", name=f"pt{step}"
                    )
                for mt in range(n_mt):
                    tp = psum_pool.tile(
                        [P, P], dt, tag=f"ps{mt}", name=f"tp{step}_{mt}"
                    )
                    nc.tensor.transpose(tp[:], yshd[:, mt, :], ident[:])
                    nc.vector.tensor_copy(
                        pt[step][:, mt * P : (mt + 1) * P], tp[:]
                    )

            def lhsT_res(kt, mt):
                q, m = divmod(mt, 2)
                return lc_res[:, q, kt, m * P : (m + 1) * P]

            # per-step structure: phases are (mt0, nmt) output slices, each
            # gathered as soon as complete; kt segments are issued
            # segment-major across phases so phase completions stagger with
            # the arrival of the previous step's gather slices
            ktA = kts_of(0, MH)
            ktB = kts_of(MH, MH)
            step_phases = {
                1: [(2 * q, 2) for q in range(4)],
                2: [(2 * q, 2) for q in range(4)],
                3: [(0, MT)],
            }
            kt_qorder = sum([kts_of(2 * q, 2) for q in range(4)], [])
            step_segs = {
                1: [list(range(KT))],
                2: [kt_qorder],
                3: [kt_qorder],
            }

            for step in (1, 2, 3):
                kt_order = list(range(KT)) if step == 1 else kt_halves

                yshd = y_pool.tile([P, MT, BF], dt, tag="yshd", name=f"yshd_{step}")
                x_nxt = None
                if step < 3:
                    x_nxt = x_pool.tile([P, KT, BF], dt, tag="x", name=f"x{step}")

                if resident:
                    segs = step_segs[step]
                    ypsum = {
                        mt: psum_pool.tile(
                            [P, BF], f32, tag=f"ps{mt}", name=f"y{step}_{mt}"
                        )
                        for mt in range(MT)
                    }
                    for si, seg in enumerate(segs):
                        first_seg = si == 0
                        last_seg = si == len(segs) - 1
                        for mt0, nmt in step_phases[step]:
                            mts = range(mt0, mt0 + nmt)
                            for ki, kt in enumerate(seg):
                                for mt in mts:
                                    nc.tensor.matmul(
                                        ypsum[mt][:],
                                        lhsT=lhsT_res(kt, mt),
                                        rhs=x_cur[:, kt, :],
                                        start=(first_seg and ki == 0),
                                        stop=(last_seg and ki == len(seg) - 1),
                                    )
                            if last_seg:
                                for mt in mts:
                                    nc.vector.tensor_copy(
                                        yshd[:, mt, :], ypsum[mt][:]
                                    )
                                if step < 3:
                                    gather_slice(step, mt0, nmt, yshd, x_nxt)
                else:
                    # fp32 fallback: stream L once per step, single phase
                    ypsum = [
                        psum_pool.tile(
                            [P, BF], f32, tag=f"ps{mt}", name=f"y{step}_{mt}"
                        )
                        for mt in range(MT)
                    ]
                    for ci in range(0, KT, LCH):
                        kts = kt_order[ci : ci + LCH]
                        lc_t = lstream_pool.tile(
                            [P, LCH, SH], dt, tag="lc_s", name=f"lc{step}_{ci}"
                        )
                        eng = nc.sync if (ci // LCH) % 2 == 0 else nc.scalar
                        if kts == list(range(kts[0], kts[0] + LCH)):
                            eng.dma_start(lc_t[:], Lc[:, kts[0] : kts[0] + LCH, :])
                        else:
                            for q, kt in enumerate(kts):
                                eng.dma_start(lc_t[:, q, :], Lc[:, kt, :])
                        for q, kt in enumerate(kts):
                            i = ci + q
                            for mt in range(MT):
                                nc.tensor.matmul(
                                    ypsum[mt][:],
                                    lhsT=lc_t[:, q, mt * P : (mt + 1) * P],
                                    rhs=x_cur[:, kt, :],
                                    start=(i == 0),
                                    stop=(i == KT - 1),
                                )
                    for mt in range(MT):
                        nc.vector.tensor_copy(yshd[:, mt, :], ypsum[mt][:])
                    if step < 3:
                        for hf in range(2):
                            gather_slice(step, hf * MH, MH, yshd, x_nxt)

                transposes(step, yshd)
                if step < 3:
                    x_cur = x_nxt

            # --- projection: outT[h] = sum_j WH[j,h].T @ P_j^T  (+ host bias) ---
            out_sb = proj_pool.tile([P, QH, 2, 512], f32, tag="out_sb")
            for h in range(QH):
                for ns in range(2):
                    pp = psum_pool.tile(
                        [P, 512], f32, tag=f"ps{h * 2 + ns}", name=f"pp{h}_{ns}"
                    )
                    for j in range(K):
                        nc.tensor.matmul(
                            pp[:],
                            lhsT=whs[:, j, h, :],
                            rhs=pt[j][:, ns * 512 : (ns + 1) * 512],
                            start=(j == 0),
                            stop=(j == K - 1),
                        )
                    nc.vector.tensor_copy(out_sb[:, h, ns, :], pp[:])
            nc.sync.dma_start(
                OUT.rearrange("h q (s n) -> q h s n", s=2), out_sb[:]
            )

    nc.compile()
    return nc


_CACHED = {}


def _get_nc(variant):
    if variant not in _CACHED:
        _CACHED[variant] = build_nc(variant)
    return _CACHED[variant]


def _prep_inputs(x, L, weight, variant):
    np_dt = _np_dt(variant)
    f32 = np.float32

    X0 = np.ascontiguousarray(
        x.astype(f32).transpose(1, 0, 2).reshape(N, BF)
    )  # [N, (b,fi)]
    X0_t = np.ascontiguousarray(
        X0.reshape(KT, P, BF).transpose(1, 0, 2)
    ).astype(np_dt)  # [P, KT, BF]
    W = weight.astype(f32)
    V = np.stack(
        [W[0] - W[2], W[1] - W[3], 2.0 * W[2], 2.0 * W[3]]
    )  # [4, F_IN, F_OUT]
    # block-diagonal packing: WH[j, h, b*F_IN+fi, bl*F_OUT+fo] = V[j,fi,fo]
    # for b == 2h + bl
    WH = np.zeros((K, QH, BF, P), dtype=f32)
    for j in range(K):
        for b in range(B):
            h, bl = divmod(b, 2)
            WH[j, h, b * F_IN : (b + 1) * F_IN, bl * F_OUT : (bl + 1) * F_OUT] = V[j]
    WH = WH.astype(np_dt)

    in_maps = []
    for r in range(NCORES):
        rows = slice(r * SH, (r + 1) * SH)
        Lc_r = np.ascontiguousarray(
            L[:, rows].reshape(KT, P, 4, SH // 4).transpose(2, 1, 0, 3)
        ).astype(np_dt)  # [4, P, KT, SH4]
        X0T_r = np.ascontiguousarray(X0[rows, :].T).astype(np_dt)
        in_maps.append({"Lc": Lc_r, "X0": X0_t, "X0T": X0T_r, "WH": WH})
    return in_maps


def _assemble(results, bias):
    out = np.empty((B, N, F_OUT), dtype=np.float32)
    for r in range(NCORES):
        outT = results[r]["OUT"]  # [QH, 128, SH]
        for b in range(B):
            h, bl = divmod(b, 2)
            out[b, r * SH : (r + 1) * SH, :] = outT[
                h, bl * F_OUT : (bl + 1) * F_OUT, :
            ].T
    out += bias.astype(np.float32)
    return out


def run(x, L, weight, bias, variant=VARIANT, trace=False):
    nc = _get_nc(variant)
    in_maps = _prep_inputs(x, L, weight, variant)
    res = bass_utils.run_bass_kernel_spmd(
        nc,
        in_maps,
        core_ids=list(range(NCORES)),
        trace=trace,
        trace_cores=list(range(NCORES)) if trace else None,
    )
    out = _assemble(res.results, bias)
    return out, res


def kernel(x, L, weight, bias):
    out, _ = run(
        np.asarray(x), np.asarray(L), np.asarray(weight), np.asarray(bias)
    )
    return out


# revision 17
# speedup vs baseline: 3.1367x; 1.1455x over previous
"""GCNCheb Trainium2 kernel: out[b,n,fo] = sum_k T_k[b,n,:] @ W[k] + bias.

T_k recurrence (matrix powers P_j = L^j x with T0=P0, T1=P1, Tk=2*P_k - T_{k-2})
is linear, so the K/F_in contraction is re-expressed over pure powers with
host-precombined weights V_j:
    out = P0 (W0-W2) + P1 (W1-W3) + P2 (2 W2) + P3 (2 W3) + bias

Distribution over 8 NeuronCores: 1D row-shard of L. Core r holds the column
slice Lc_r = L[:, r*1024:(r+1)*1024] (== L[rows_r,:].T since L is symmetric),
pre-tiled on host to [128, 64, 1024] so every DMA is contiguous per partition.
X is [N, B*F_in] = [8192, 128] (batch folded into columns), pre-tiled to
[128, 64, 128]. Each step every core computes its 1024 rows of X_next = L @ X
via 64 k-tile PSUM accumulations, then an AllGather rebuilds the full X.

Latency structure: steps 1 and 2 are split into two m-half phases; each
half-shard is gathered as soon as its phase completes, so the collective's
firmware latency and inter-core skew hide behind the other half's matmuls.
Consuming steps issue k-tiles of gather-half A before half B. The final output
is gathered host-side from per-core results.

The projection runs on-device on the core's row shard: P_j^T tiles (built with
PE transposes) are contracted with block-diagonal weights packing all 4 batches,
producing out^T per core; the host untangles layout and adds bias.

bf16 variant (default): L is bf16 and fully SBUF-resident (16 MB), single DMA
pass split across both HWDGE queues (sync + scalar); projection path in bf16.
"""

import os
import sys

sys.path.insert(0, "/opt/trn_rl_repo")

import numpy as np

import concourse.bass as bass
import concourse.mybir as mybir
import concourse.tile as tile
from concourse import bacc, bass_utils
from concourse.masks import make_identity

B, N, F_IN, F_OUT, K = 4, 8192, 32, 64, 4
NCORES = 8
P = 128
SH = N // NCORES          # rows per core (1024)
BF = B * F_IN             # folded X columns (128)
KT = N // P               # contraction tiles (64)
MT = SH // P              # output row tiles per core (8)
MH = MT // 2              # half-shard m-tiles (4)
QH = 2                    # output halves: (b in {2h, 2h+1}) x F_OUT = 128 partitions

VARIANT = os.environ.get("GCN_VARIANT", "bf16")

_DT = {"bf16": mybir.dt.bfloat16, "fp32": mybir.dt.float32}


def _np_dt(variant):
    if variant == "bf16":
        import ml_dtypes

        return np.dtype(ml_dtypes.bfloat16)
    return np.dtype(np.float32)


def build_nc(variant=VARIANT):
    dt = _DT[variant]
    f32 = mybir.dt.float32
    resident = variant == "bf16"  # 16 MB bf16 L fits in SBUF

    nc = bacc.Bacc()
    # all pre-tiled on host: partition-major, fully contiguous per partition
    Lc = nc.dram_tensor("Lc", [4, P, KT, SH // 4], dt, kind="ExternalInput")
    X0 = nc.dram_tensor("X0", [P, KT, BF], dt, kind="ExternalInput")
    X0T = nc.dram_tensor("X0T", [BF, SH], dt, kind="ExternalInput")
    WH = nc.dram_tensor("WH", [K, QH, BF, P], dt, kind="ExternalInput")
    OUT = nc.dram_tensor("OUT", [QH, P, SH], f32, kind="ExternalOutput")

    # k-tile consumption orders matching how the previous step's gathers land
    def kts_of(mt0, nmt):
        return [r * MT + mt0 + m for r in range(NCORES) for m in range(nmt)]

    kt_quarters = sum([kts_of(2 * q, 2) for q in range(4)], [])
    kt_halves = kts_of(0, MH) + kts_of(MH, MH)

    with tile.TileContext(nc) as tc:
        with (
            tc.tile_pool(name="lres", bufs=1) as lres_pool,
            tc.tile_pool(name="lstream", bufs=6) as lstream_pool,
            tc.tile_pool(name="xbuf", bufs=1) as x_pool,
            tc.tile_pool(name="ybuf", bufs=2) as y_pool,
            tc.tile_pool(name="proj", bufs=1) as proj_pool,
            tc.tile_pool(name="psum", bufs=1, space="PSUM") as psum_pool,
            tc.tile_pool(name="dram", bufs=1, space="DRAM") as dram_pool,
        ):
            # --- initial loads: X first (everything waits on it), L chunks
            # alternating across the two HWDGE queues (sync + scalar) ---
            x_cur = x_pool.tile([P, KT, BF], dt, tag="x", name="x0")
            half = KT // 2
            nc.sync.dma_start(x_cur[:, :half, :], X0[:, :half, :])
            nc.sync.dma_start(x_cur[:, half:, :], X0[:, half:, :])

            LCH = 4  # k-tiles per L DMA chunk (fp32 streaming path)
            SH4 = SH // 4
            if resident:
                lc_res = lres_pool.tile([P, 4, KT, SH4], dt, tag="lc_res")
                for q in range(4):
                    for ko in range(0, KT, 8):
                        nc.scalar.dma_start(
                            lc_res[:, q, ko : ko + 8, :],
                            Lc[q, :, ko : ko + 8, :],
                        )

            whs = proj_pool.tile([P, K, QH, P], dt, tag="whs")
            nc.sync.dma_start(whs[:], WH.rearrange("k h p m -> p k h m"))
            pt0 = proj_pool.tile([P, SH], dt, tag="pt0")
            nc.sync.dma_start(pt0[:], X0T[:, :])
            ident = proj_pool.tile([P, P], dt, tag="ident")
            make_identity(nc, ident[:])

            pt = [pt0, None, None, None]
            out_sb = proj_pool.tile([P, QH, 2, 512], f32, tag="out_sb")

            def gather_slice(step, mt0, nmt, yshd, x_nxt):
                """DMA mt-slice of the shard out, AllGather it, DMA back."""
                shard = dram_pool.tile([P, nmt, BF], dt, name=f"shard{step}_{mt0}")
                full = dram_pool.tile(
                    [NCORES * P, nmt, BF],
                    dt,
                    addr_space="Shared",
                    name=f"full{step}_{mt0}",
                )
                nc.sync.dma_start(shard.opt(), yshd[:, mt0 : mt0 + nmt, :])
                nc.gpsimd.collective_compute(
                    "AllGather",
                    mybir.AluOpType.bypass,
                    replica_groups=[list(range(NCORES))],
                    ins=[shard.opt()],
                    outs=[full.opt()],
                )
                xv = x_nxt[:].rearrange("p (r mt) f -> p r mt f", r=NCORES)
                nc.scalar.dma_start(
                    xv[:, :, mt0 : mt0 + nmt, :],
                    full[:].rearrange("(r p) mt f -> p r mt f", p=P),
                )

            def transposes(step, yshd, n_mt=MT):
                """PE-transpose the row shard into P_j^T [BF, SH] for projection."""
                if pt[step] is None:
                    pt[step] = proj_pool.tile(
                        [P, SH], dt, tag=f"pt{step}", name=f"pt{step}"
                    )
                for mt in range(n_mt):
                    tp = psum_pool.tile(
                        [P, P], dt, tag=f"ps{mt}", name=f"tp{step}_{mt}"
                    )
                    nc.tensor.transpose(tp[:], yshd[:, mt, :], ident[:])
                    nc.vector.tensor_copy(
                        pt[step][:, mt * P : (mt + 1) * P], tp[:]
                    )

            def lhsT_res(kt, mt):
                q, m = divmod(mt, 2)
                return lc_res[:, q, kt, m * P : (m + 1) * P]

            # per-step structure: phases are (mt0, nmt) output slices, each
            # gathered as soon as complete; kt segments are issued
            # segment-major across phases so phase completions stagger with
            # the arrival of the previous step's gather slices
            ktA = kts_of(0, MH)
            ktB = kts_of(MH, MH)
            step_phases = {
                1: [(2 * q, 2) for q in range(4)],
                2: [(2 * q, 2) for q in range(4)],
                3: [(0, MT)],
            }
            kt_qorder = sum([kts_of(2 * q, 2) for q in range(4)], [])
            step_segs = {
                1: [list(range(KT))],
                2: [kt_qorder],
                3: [kt_qorder],
            }

            for step in (1, 2, 3):
                kt_order = list(range(KT)) if step == 1 else kt_halves

                yshd = y_pool.tile([P, MT, BF], dt, tag="yshd", name=f"yshd_{step}")
                x_nxt = None
                if step < 3:
                    x_nxt = x_pool.tile([P, KT, BF], dt, tag="x", name=f"x{step}")

                if resident and step == 3:
                    pt3 = proj_pool.tile([P, SH], dt, tag="pt3", name="pt3")
                    pt[3] = pt3
                    seg = step_segs[3][0]
                    for ns in range(2):
                        pp3 = psum_pool.tile(
                            [P, 512], f32, tag=f"ps{ns * 4}", name=f"p3t_{ns}"
                        )
                        for ki, kt in enumerate(seg):
                            nc.tensor.matmul(
                                pp3[:],
                                lhsT=x_cur[:, kt, :],
                                rhs=lc_res[:, 2 * ns : 2 * ns + 2, kt, :],
                                start=(ki == 0),
                                stop=(ki == len(seg) - 1),
                            )
                        nc.vector.tensor_copy(
                            pt3[:, ns * 512 : (ns + 1) * 512], pp3[:]
                        )
                        # project this ns column-half right away
                        for h in range(QH):
                            pp = psum_pool.tile(
                                [P, 512],
                                f32,
                                tag=f"ps{ns * 4 + 1 + h}",
                                name=f"pp{h}_{ns}",
                            )
                            for j in range(K):
                                nc.tensor.matmul(
                                    pp[:],
                                    lhsT=whs[:, j, h, :],
                                    rhs=pt[j][:, ns * 512 : (ns + 1) * 512],
                                    start=(j == 0),
                                    stop=(j == K - 1),
                                )
                            nc.vector.tensor_copy(out_sb[:, h, ns, :], pp[:])
                    continue

                if resident:
                    segs = step_segs[step]
                    ypsum = {
                        mt: psum_pool.tile(
                            [P, BF], f32, tag=f"ps{mt}", name=f"y{step}_{mt}"
                        )
                        for mt in range(MT)
                    }
                    for si, seg in enumerate(segs):
                        first_seg = si == 0
                        last_seg = si == len(segs) - 1
                        for mt0, nmt in step_phases[step]:
                            mts = range(mt0, mt0 + nmt)
                            for ki, kt in enumerate(seg):
                                for mt in mts:
                                    nc.tensor.matmul(
                                        ypsum[mt][:],
                                        lhsT=lhsT_res(kt, mt),
                                        rhs=x_cur[:, kt, :],
                                        start=(first_seg and ki == 0),
                                        stop=(last_seg and ki == len(seg) - 1),
                                    )
                            if last_seg:
                                for mt in mts:
                                    nc.vector.tensor_copy(
                                        yshd[:, mt, :], ypsum[mt][:]
                                    )
                                if step < 3:
                                    gather_slice(step, mt0, nmt, yshd, x_nxt)
                else:
                    # fp32 fallback: stream L once per step, single phase
                    ypsum = [
                        psum_pool.tile(
                            [P, BF], f32, tag=f"ps{mt}", name=f"y{step}_{mt}"
                        )
                        for mt in range(MT)
                    ]
                    for ci in range(0, KT, LCH):
                        kts = kt_order[ci : ci + LCH]
                        lc_t = lstream_pool.tile(
                            [P, LCH, SH], dt, tag="lc_s", name=f"lc{step}_{ci}"
                        )
                        eng = nc.sync if (ci // LCH) % 2 == 0 else nc.scalar
                        if kts == list(range(kts[0], kts[0] + LCH)):
                            eng.dma_start(lc_t[:], Lc[:, kts[0] : kts[0] + LCH, :])
                        else:
                            for q, kt in enumerate(kts):
                                eng.dma_start(lc_t[:, q, :], Lc[:, kt, :])
                        for q, kt in enumerate(kts):
                            i = ci + q
                            for mt in range(MT):
                                nc.tensor.matmul(
                                    ypsum[mt][:],
                                    lhsT=lc_t[:, q, mt * P : (mt + 1) * P],
                                    rhs=x_cur[:, kt, :],
                                    start=(i == 0),
                                    stop=(i == KT - 1),
                                )
                    for mt in range(MT):
                        nc.vector.tensor_copy(yshd[:, mt, :], ypsum[mt][:])
                    if step < 3:
                        for hf in range(2):
                            gather_slice(step, hf * MH, MH, yshd, x_nxt)

                if pt[step] is None:
                    transposes(step, yshd)
                if step < 3:
                    x_cur = x_nxt

            # --- projection (fp32 streaming fallback; resident path projects
            # inside step 3) ---
            if not resident:
              for h in range(QH):
                  for ns in range(2):
                    pp = psum_pool.tile(
                        [P, 512], f32, tag=f"ps{h * 2 + ns}", name=f"ppf{h}_{ns}"
                    )
                    for j in range(K):
                        nc.tensor.matmul(
                            pp[:],
                            lhsT=whs[:, j, h, :],
                            rhs=pt[j][:, ns * 512 : (ns + 1) * 512],
                            start=(j == 0),
                            stop=(j == K - 1),
                        )
                    nc.vector.tensor_copy(out_sb[:, h, ns, :], pp[:])
            nc.sync.dma_start(
                OUT.rearrange("h q (s n) -> q h s n", s=2), out_sb[:]
            )

    nc.compile()
    return nc


_CACHED = {}


def _get_nc(variant):
    if variant not in _CACHED:
        _CACHED[variant] = build_nc(variant)
    return _CACHED[variant]


def _prep_inputs(x, L, weight, variant):
    np_dt = _np_dt(variant)
    f32 = np.float32

    X0 = np.ascontiguousarray(
        x.astype(f32).transpose(1, 0, 2).reshape(N, BF)
    )  # [N, (b,fi)]
    X0_t = np.ascontiguousarray(
        X0.reshape(KT, P, BF).transpose(1, 0, 2)
    ).astype(np_dt)  # [P, KT, BF]
    W = weight.astype(f32)
    V = np.stack(
        [W[0] - W[2], W[1] - W[3], 2.0 * W[2], 2.0 * W[3]]
    )  # [4, F_IN, F_OUT]
    # block-diagonal packing: WH[j, h, b*F_IN+fi, bl*F_OUT+fo] = V[j,fi,fo]
    # for b == 2h + bl
    WH = np.zeros((K, QH, BF, P), dtype=f32)
    for j in range(K):
        for b in range(B):
            h, bl = divmod(b, 2)
            WH[j, h, b * F_IN : (b + 1) * F_IN, bl * F_OUT : (bl + 1) * F_OUT] = V[j]
    WH = WH.astype(np_dt)

    in_maps = []
    for r in range(NCORES):
        rows = slice(r * SH, (r + 1) * SH)
        Lc_r = np.ascontiguousarray(
            L[:, rows].reshape(KT, P, 4, SH // 4).transpose(2, 1, 0, 3)
        ).astype(np_dt)  # [4, P, KT, SH4]
        X0T_r = np.ascontiguousarray(X0[rows, :].T).astype(np_dt)
        in_maps.append({"Lc": Lc_r, "X0": X0_t, "X0T": X0T_r, "WH": WH})
    return in_maps


def _assemble(results, bias):
    out = np.empty((B, N, F_OUT), dtype=np.float32)
    for r in range(NCORES):
        outT = results[r]["OUT"]  # [QH, 128, SH]
        for b in range(B):
            h, bl = divmod(b, 2)
            out[b, r * SH : (r + 1) * SH, :] = outT[
                h, bl * F_OUT : (bl + 1) * F_OUT, :
            ].T
    out += bias.astype(np.float32)
    return out


def run(x, L, weight, bias, variant=VARIANT, trace=False):
    nc = _get_nc(variant)
    in_maps = _prep_inputs(x, L, weight, variant)
    last_err = None
    for attempt in range(3):
        try:
            res = bass_utils.run_bass_kernel_spmd(
                nc,
                in_maps,
                core_ids=list(range(NCORES)),
                trace=trace,
                trace_cores=list(range(NCORES)) if trace else None,
            )
            break
        except Exception as e:  # transient device wedge: retry
            last_err = e
            import time

            time.sleep(10)
    else:
        raise last_err
    out = _assemble(res.results, bias)
    return out, res


def kernel(x, L, weight, bias):
    out, _ = run(
        np.asarray(x), np.asarray(L), np.asarray(weight), np.asarray(bias)
    )
    return out
